# revision 10
# baseline (speedup 1.0000x reference)
"""Trainium2 Bass kernel: 2D valid cross-correlation (4096x4096 image, 15x15 kernel).

Strategy: shard output COLUMNS across 8 NeuronCores (spatial data-parallel, no
device-to-device communication). Each core computes the full 4082 output rows
for its 512 output columns.

On-core formulation (space-to-depth / patch matmul): the image is tiled into
8x16 = 128-pixel patches. Host-side, X is repacked so SBUF partition k holds
within-patch position k = 8*r + c... actually k = r*16 + c, and the free
dimension enumerates patches p = q*33 + s (33 column-patches per patch-row,
including halo). An output patch (8x16 = 128 pixels, the matmul M dim) draws
its 15x15 receptive field from exactly 6 input patches: row offsets dq in
{0,1,2} x col offsets ds in {0,1}. For each (dq, ds) the map from input-patch
pixels to output-patch pixels is a fixed DENSE 128x128 matrix
  S[(rk,ck),(rm,cm)] = w[8*dq + rk - rm, 16*ds + ck - cm]  (0 where out of range)
so the whole conv is 6 PSUM-accumulated bf16 matmuls per 512-patch chunk, with
the moving operand a contiguous slice of the s2d buffer shifted by dq*33 + ds.
This streams ~6 PE columns per 128 outputs vs ~15 per 114 for a banded-Toeplitz
formulation (2.9x fewer cycles), uses the full 128x128 array, and bf16 enables
the automatic fast-weight-load path (fp32r cannot), hiding stationary reloads.

bf16 inputs give ~1e-3 relative error vs the fp32 reference (tolerance 2e-2):
products are exact in the PE, accumulation is fp32 in PSUM.
"""

import numpy as np
import ml_dtypes

import concourse.bass as bass
import concourse.mybir as mybir
import concourse.tile as tile
from concourse import bacc
from concourse.bass_utils import run_bass_kernel_spmd

H, W = 4096, 4096
KH, KW = 15, 15
OH, OW = H - KH + 1, W - KW + 1  # 4082 x 4082

NCORES = 8
COLS_PER_CORE = 512               # output cols per core (core 7: 498 valid)
IN_COLS = COLS_PER_CORE + KW - 1  # 526 input cols (with halo)

PR, PC = 8, 16                    # patch = 8 rows x 16 cols = 128 = PE contraction
SC = 33                           # col-patches per patch-row (528 cols incl. pad)
QI = 514                          # input patch-rows (4096 rows + 16 pad = 4112)
QO = 511                          # output patch-rows (4082 = 510*8 + 2)
NPI = QI * SC                     # 16962 input patches
NPO = QO * SC                     # 16863 real output patches (s=32 col is garbage)
NPO_DEV = 16864                   # padded so the last chunk has even N (480)

NT = 512                          # output patches per chunk = one fp32 PSUM bank
NCHUNKS = 33                      # 32 x 512 + 480
PASSES = [(0, 0), (0, 1), (1, 0), (1, 1), (2, 0), (2, 1)]

# Input slabs (patch ranges incl. the +66+1 halo read span of their chunks).
# Small first slab so the first matmul isn't gated on a long descriptor-gen;
# each HWDGE queue generates descriptors at only ~145 GB/s. Slab 1 goes on the
# scalar queue (parallel with slab 0 on sync) so the head of the stream never
# outruns the input. slab k covers chunks [SLAB_CHUNKS[k], SLAB_CHUNKS[k+1])
SLAB_CHUNKS = [0, 3, 8, 16, 24, 33]
SLABW_MAX = 4643

F32 = mybir.dt.float32
BF16 = mybir.dt.bfloat16

# Output DMA plan: (first_chunk, n_chunks, queue). DMA-capable queues are only
# sync (SP), scalar (Activation) and gpsimd (SWDGE, ~2us fixed setup). Bytes
# per queue are the binding constraint (~145 GB/s descriptor-gen per queue),
# so early chunks go out in 4-chunk groups, gpsimd takes mid groups, and the
# tail shrinks to singles alternating scalar/sync (sync is free after the
# input slabs finish ~32us in).
OUT_PLAN = [
    (0, 4, "scalar"),
    (4, 4, "gpsimd"),
    (8, 4, "scalar"),
    (12, 4, "gpsimd"),
    (16, 4, "scalar"),
    (20, 2, "scalar"),
    (22, 2, "sync"),
    (24, 2, "sync"),
    (26, 2, "gpsimd"),
    (28, 1, "scalar"),
    (29, 1, "sync"),
    (30, 1, "scalar"),
    (31, 1, "sync"),
    (32, 1, "split"),  # final chunk: halve across scalar+sync for min tail
]


def _build_program():
    nc = bacc.Bacc("TRN2", target_bir_lowering=False, debug=False)
    x = nc.dram_tensor("x", [128, NPI], BF16, kind="ExternalInput").ap()
    wt = nc.dram_tensor("wt", [128, 6 * 128], BF16, kind="ExternalInput").ap()
    out = nc.dram_tensor("out", [128, NPO_DEV], F32, kind="ExternalOutput").ap()

    chunk_group = {}  # chunk -> (first_chunk, n_chunks, queue)
    for g in OUT_PLAN:
        for c in range(g[0], g[0] + g[1]):
            chunk_group[c] = g

    # slab index for a chunk
    def slab_of(c):
        for k in range(len(SLAB_CHUNKS) - 1):
            if SLAB_CHUNKS[k] <= c < SLAB_CHUNKS[k + 1]:
                return k
        raise AssertionError(c)

    with tile.TileContext(nc) as tc:
        with (
            tc.tile_pool(name="wpool", bufs=1) as wpool,
            tc.tile_pool(name="xpool", bufs=3) as xpool,
            tc.tile_pool(name="opool", bufs=4) as opool,
            tc.tile_pool(name="dpool", bufs=1) as dpool,
            tc.tile_pool(name="ppool", bufs=7, space="PSUM") as ppool,
            tc.tile_pool(name="dps", bufs=1, space="PSUM") as dps,
        ):
            # Input slab 0 first: its descriptor-gen gates the first real
            # matmul. wt in parallel on the scalar queue.
            slabs = {}

            def start_slab(k, queue=None):
                p_lo = NT * SLAB_CHUNKS[k]
                p_hi = min(NT * SLAB_CHUNKS[k + 1], NPO_DEV) + 2 * SC + 1
                st = xpool.tile([128, SLABW_MAX], BF16, tag="xs")
                (queue or nc.sync).dma_start(st[:, : p_hi - p_lo], x[:, p_lo:p_hi])
                slabs[k] = st

            start_slab(0)
            wtile = wpool.tile([128, 6 * 128], BF16, tag="wt")
            nc.scalar.dma_start(wtile[:], wt[:])
            start_slab(1, queue=nc.scalar)

            # HAM pre-warm: the PE clock-gate only opens to 2.4GHz after
            # ~3.4us of sustained activity. Run dummy matmuls (no input
            # deps) while slab 0's DMA lands so the real matmuls start
            # near full clock.
            dz = dpool.tile([128, 128], F32, tag="dz")
            nc.vector.memset(dz[:], 0)
            dummy = dpool.tile([128, 128], BF16, tag="dummy")
            nc.vector.tensor_copy(dummy[:], dz[:])
            dacc = dps.tile([128, 128], F32)
            for _ in range(17):
                nc.tensor.matmul(dacc[:], dummy[:], dummy[:], start=True, stop=True)

            dma_q = {
                "sync": nc.sync,
                "scalar": nc.scalar,
                "gpsimd": nc.gpsimd,
            }
            stage = None
            for c in range(NCHUNKS):
                k = slab_of(c)
                if k + 1 not in slabs and k + 1 < len(SLAB_CHUNKS) - 1:
                    start_slab(k + 1)
                st = slabs[k]
                p0 = NT * c
                n = min(NT, NPO_DEV - p0)
                base = p0 - NT * SLAB_CHUNKS[k]
                acc = ppool.tile([128, NT], F32)
                for i, (dq, ds) in enumerate(PASSES):
                    off = base + dq * SC + ds
                    nc.tensor.matmul(
                        acc[:, :n],
                        wtile[:, i * 128 : (i + 1) * 128],
                        st[:, off : off + n],
                        start=(i == 0),
                        stop=(i == 5),
                    )
                g0, gn_chunks, q = chunk_group[c]
                gi = c - g0
                if gi == 0:
                    stage = opool.tile([128, 4 * NT], F32, tag="ot")
                nc.vector.tensor_copy(stage[:, gi * NT : gi * NT + n], acc[:, :n])
                if gi == gn_chunks - 1:
                    gp0 = NT * g0
                    gn = p0 + n - gp0
                    if q == "split":
                        h = (gn // 2 + 1) & ~1
                        nc.scalar.dma_start(out[:, gp0 : gp0 + h], stage[:, :h])
                        nc.sync.dma_start(
                            out[:, gp0 + h : gp0 + gn], stage[:, h:gn]
                        )
                    else:
                        dma_q[q].dma_start(out[:, gp0 : gp0 + gn], stage[:, :gn])
    nc.finalize()
    return nc


def _pack_weights(weight: np.ndarray) -> np.ndarray:
    """6 dense 128x128 stationary matrices (one per (dq, ds) pass)."""
    wt = np.zeros((128, 6 * 128), dtype=np.float32)
    rk, ck = np.divmod(np.arange(128)[:, None], PC)  # input pixel within patch
    rm, cm = np.divmod(np.arange(128)[None, :], PC)  # output pixel within patch
    for i, (dq, ds) in enumerate(PASSES):
        a = PR * dq + rk - rm
        b = PC * ds + ck - cm
        valid = (a >= 0) & (a < KH) & (b >= 0) & (b < KW)
        wt[:, i * 128 : (i + 1) * 128] = np.where(
            valid, weight[np.clip(a, 0, KH - 1), np.clip(b, 0, KW - 1)], 0.0
        )
    return wt


def kernel(X: np.ndarray, weight: np.ndarray, bias: np.ndarray) -> np.ndarray:
    X = np.ascontiguousarray(X, dtype=np.float32)
    weight = np.ascontiguousarray(weight, dtype=np.float32)
    bias = np.asarray(bias, dtype=np.float32)

    wt = _pack_weights(weight).astype(ml_dtypes.bfloat16)

    in_maps = []
    for c in range(NCORES):
        xs = np.zeros((QI * PR, SC * PC), dtype=np.float32)
        c0 = c * COLS_PER_CORE
        c1 = min(c0 + IN_COLS, W)
        xs[:H, : c1 - c0] = X[:, c0:c1]
        x_s2d = (
            xs.reshape(QI, PR, SC, PC)
            .transpose(1, 3, 0, 2)
            .reshape(128, NPI)
            .astype(ml_dtypes.bfloat16)
        )
        in_maps.append({"x": np.ascontiguousarray(x_s2d), "wt": wt})

    nc = _build_program()
    res = run_bass_kernel_spmd(nc, in_maps, core_ids=list(range(NCORES)))
    global _last_results
    _last_results = res

    out = np.empty((OH, OW), dtype=np.float32)
    for c in range(NCORES):
        o = np.asarray(res.results[c]["out"], dtype=np.float32)[:, :NPO]
        o4 = o.reshape(PR, PC, QO, SC).transpose(2, 0, 3, 1).reshape(QO * PR, SC * PC)
        c0 = c * COLS_PER_CORE
        n = min(COLS_PER_CORE, OW - c0)
        out[:, c0 : c0 + n] = o4[:OH, :n]

    b0 = float(bias.reshape(-1)[0]) if bias.size else 0.0
    if b0 != 0.0:
        out += b0
    return out


# revision 11
# speedup vs baseline: 1.1673x; 1.1673x over previous
"""Trainium2 Bass kernel: 2D valid cross-correlation (4096x4096 image, 15x15 kernel).

Strategy: shard output COLUMNS across 8 NeuronCores (spatial data-parallel, no
device-to-device communication). Each core computes the full 4082 output rows
for its 512 output columns.

On-core formulation (space-to-depth / patch matmul): the image is tiled into
8x16 = 128-pixel patches. Host-side, X is repacked so SBUF partition k holds
within-patch position k = 8*r + c... actually k = r*16 + c, and the free
dimension enumerates patches p = q*33 + s (33 column-patches per patch-row,
including halo). An output patch (8x16 = 128 pixels, the matmul M dim) draws
its 15x15 receptive field from exactly 6 input patches: row offsets dq in
{0,1,2} x col offsets ds in {0,1}. For each (dq, ds) the map from input-patch
pixels to output-patch pixels is a fixed DENSE 128x128 matrix
  S[(rk,ck),(rm,cm)] = w[8*dq + rk - rm, 16*ds + ck - cm]  (0 where out of range)
so the whole conv is 6 PSUM-accumulated bf16 matmuls per 512-patch chunk, with
the moving operand a contiguous slice of the s2d buffer shifted by dq*33 + ds.
This streams ~6 PE columns per 128 outputs vs ~15 per 114 for a banded-Toeplitz
formulation (2.9x fewer cycles), uses the full 128x128 array, and bf16 enables
the automatic fast-weight-load path (fp32r cannot), hiding stationary reloads.

bf16 inputs give ~1e-3 relative error vs the fp32 reference (tolerance 2e-2):
products are exact in the PE, accumulation is fp32 in PSUM.
"""

import numpy as np
import ml_dtypes

import concourse.bass as bass
import concourse.mybir as mybir
import concourse.tile as tile
from concourse import bacc
from concourse.bass_utils import run_bass_kernel_spmd

H, W = 4096, 4096
KH, KW = 15, 15
OH, OW = H - KH + 1, W - KW + 1  # 4082 x 4082

NCORES = 8
COLS_PER_CORE = 512               # output cols per core (core 7: 498 valid)
IN_COLS = COLS_PER_CORE + KW - 1  # 526 input cols (with halo)

PR, PC = 8, 16                    # patch = 8 rows x 16 cols = 128 = PE contraction
SC = 33                           # col-patches per patch-row (528 cols incl. pad)
QI = 514                          # input patch-rows (4096 rows + 16 pad = 4112)
QO = 511                          # output patch-rows (4082 = 510*8 + 2)
NPI = QI * SC                     # 16962 input patches
NPO = QO * SC                     # 16863 real output patches (s=32 col is garbage)
NPO_DEV = 16864                   # padded so the last chunk has even N (480)

NT = 512                          # output patches per chunk = one fp32 PSUM bank
NCHUNKS = 33                      # 32 x 512 + 480
PASSES = [(0, 0), (0, 1), (1, 0), (1, 1), (2, 0), (2, 1)]

# Input slabs (patch ranges incl. the +66+1 halo read span of their chunks).
# Small first slab so the first matmul isn't gated on a long descriptor-gen;
# each HWDGE queue generates descriptors at only ~145 GB/s. Slab 1 goes on the
# scalar queue (parallel with slab 0 on sync) so the head of the stream never
# outruns the input. slab k covers chunks [SLAB_CHUNKS[k], SLAB_CHUNKS[k+1])
SLAB_CHUNKS = [0, 3, 8, 16, 24, 33]
SLABW_MAX = 4643

F32 = mybir.dt.float32
BF16 = mybir.dt.bfloat16

# Output DMA plan: (first_chunk, n_chunks, queue). DMA-capable queues are only
# sync (SP), scalar (Activation) and gpsimd (SWDGE, ~2us fixed setup). Bytes
# per queue are the binding constraint (~145 GB/s descriptor-gen per queue),
# so early chunks go out in 4-chunk groups, gpsimd takes mid groups, and the
# tail shrinks to singles alternating scalar/sync (sync is free after the
# input slabs finish ~32us in).
OUT_PLAN = [
    (0, 4, "scalar"),
    (4, 4, "gpsimd"),
    (8, 4, "scalar"),
    (12, 4, "gpsimd"),
    (16, 4, "scalar"),
    (20, 2, "scalar"),
    (22, 2, "sync"),
    (24, 2, "sync"),
    (26, 2, "gpsimd"),
    (28, 1, "scalar"),
    (29, 1, "sync"),
    (30, 1, "scalar"),
    (31, 1, "sync"),
    (32, 1, "split"),  # final chunk: halve across scalar+sync for min tail
]


def _build_program():
    nc = bacc.Bacc("TRN2", target_bir_lowering=False, debug=False)
    x = nc.dram_tensor("x", [128, NPI], BF16, kind="ExternalInput").ap()
    wt = nc.dram_tensor("wt", [128, 6 * 128], BF16, kind="ExternalInput").ap()
    out = nc.dram_tensor("out", [128, NPO_DEV], F32, kind="ExternalOutput").ap()

    chunk_group = {}  # chunk -> (first_chunk, n_chunks, queue)
    for g in OUT_PLAN:
        for c in range(g[0], g[0] + g[1]):
            chunk_group[c] = g

    # slab index for a chunk
    def slab_of(c):
        for k in range(len(SLAB_CHUNKS) - 1):
            if SLAB_CHUNKS[k] <= c < SLAB_CHUNKS[k + 1]:
                return k
        raise AssertionError(c)

    with tile.TileContext(nc) as tc:
        with (
            tc.tile_pool(name="wpool", bufs=1) as wpool,
            tc.tile_pool(name="xpool", bufs=3) as xpool,
            tc.tile_pool(name="opool", bufs=4) as opool,
            tc.tile_pool(name="dpool", bufs=1) as dpool,
            tc.tile_pool(name="ppool", bufs=7, space="PSUM") as ppool,
            tc.tile_pool(name="dps", bufs=1, space="PSUM") as dps,
        ):
            # Input slab 0 first: its descriptor-gen gates the first real
            # matmul. wt in parallel on the scalar queue.
            slabs = {}

            def start_slab(k, queue=None):
                p_lo = NT * SLAB_CHUNKS[k]
                p_hi = min(NT * SLAB_CHUNKS[k + 1], NPO_DEV) + 2 * SC + 1
                st = xpool.tile([128, SLABW_MAX], BF16, tag="xs")
                (queue or nc.sync).dma_start(st[:, : p_hi - p_lo], x[:, p_lo:p_hi])
                slabs[k] = st

            start_slab(0)
            wtile = wpool.tile([128, 6 * 128], BF16, tag="wt")
            nc.scalar.dma_start(wtile[:], wt[:])
            start_slab(1, queue=nc.scalar)

            # HAM pre-warm: the PE clock-gate only opens to 2.4GHz after
            # ~3.4us of sustained activity. Run dummy matmuls (no input
            # deps) while slab 0's DMA lands so the real matmuls start
            # near full clock.
            dz = dpool.tile([128, 128], F32, tag="dz")
            nc.vector.memset(dz[:], 0)
            dummy = dpool.tile([128, 128], BF16, tag="dummy")
            nc.vector.tensor_copy(dummy[:], dz[:])
            dacc = dps.tile([128, 128], F32)
            for _ in range(24):
                nc.tensor.matmul(dacc[:], dummy[:], dummy[:], start=True, stop=True)

            dma_q = {
                "sync": nc.sync,
                "scalar": nc.scalar,
                "gpsimd": nc.gpsimd,
            }
            stage = None
            for c in range(NCHUNKS):
                k = slab_of(c)
                if k + 1 not in slabs and k + 1 < len(SLAB_CHUNKS) - 1:
                    start_slab(k + 1)
                st = slabs[k]
                p0 = NT * c
                n = min(NT, NPO_DEV - p0)
                base = p0 - NT * SLAB_CHUNKS[k]
                acc = ppool.tile([128, NT], F32)
                for i, (dq, ds) in enumerate(PASSES):
                    off = base + dq * SC + ds
                    nc.tensor.matmul(
                        acc[:, :n],
                        wtile[:, i * 128 : (i + 1) * 128],
                        st[:, off : off + n],
                        start=(i == 0),
                        stop=(i == 5),
                    )
                g0, gn_chunks, q = chunk_group[c]
                gi = c - g0
                if gi == 0:
                    stage = opool.tile([128, 4 * NT], F32, tag="ot")
                nc.vector.tensor_copy(stage[:, gi * NT : gi * NT + n], acc[:, :n])
                if gi == gn_chunks - 1:
                    gp0 = NT * g0
                    gn = p0 + n - gp0
                    if q == "split":
                        h = (gn // 2 + 1) & ~1
                        nc.scalar.dma_start(out[:, gp0 : gp0 + h], stage[:, :h])
                        nc.sync.dma_start(
                            out[:, gp0 + h : gp0 + gn], stage[:, h:gn]
                        )
                    else:
                        dma_q[q].dma_start(out[:, gp0 : gp0 + gn], stage[:, :gn])
    nc.finalize()
    return nc


def _pack_weights(weight: np.ndarray) -> np.ndarray:
    """6 dense 128x128 stationary matrices (one per (dq, ds) pass)."""
    wt = np.zeros((128, 6 * 128), dtype=np.float32)
    rk, ck = np.divmod(np.arange(128)[:, None], PC)  # input pixel within patch
    rm, cm = np.divmod(np.arange(128)[None, :], PC)  # output pixel within patch
    for i, (dq, ds) in enumerate(PASSES):
        a = PR * dq + rk - rm
        b = PC * ds + ck - cm
        valid = (a >= 0) & (a < KH) & (b >= 0) & (b < KW)
        wt[:, i * 128 : (i + 1) * 128] = np.where(
            valid, weight[np.clip(a, 0, KH - 1), np.clip(b, 0, KW - 1)], 0.0
        )
    return wt


def kernel(X: np.ndarray, weight: np.ndarray, bias: np.ndarray) -> np.ndarray:
    X = np.ascontiguousarray(X, dtype=np.float32)
    weight = np.ascontiguousarray(weight, dtype=np.float32)
    bias = np.asarray(bias, dtype=np.float32)

    wt = _pack_weights(weight).astype(ml_dtypes.bfloat16)

    in_maps = []
    for c in range(NCORES):
        xs = np.zeros((QI * PR, SC * PC), dtype=np.float32)
        c0 = c * COLS_PER_CORE
        c1 = min(c0 + IN_COLS, W)
        xs[:H, : c1 - c0] = X[:, c0:c1]
        x_s2d = (
            xs.reshape(QI, PR, SC, PC)
            .transpose(1, 3, 0, 2)
            .reshape(128, NPI)
            .astype(ml_dtypes.bfloat16)
        )
        in_maps.append({"x": np.ascontiguousarray(x_s2d), "wt": wt})

    nc = _build_program()
    res = run_bass_kernel_spmd(nc, in_maps, core_ids=list(range(NCORES)))
    global _last_results
    _last_results = res

    out = np.empty((OH, OW), dtype=np.float32)
    for c in range(NCORES):
        o = np.asarray(res.results[c]["out"], dtype=np.float32)[:, :NPO]
        o4 = o.reshape(PR, PC, QO, SC).transpose(2, 0, 3, 1).reshape(QO * PR, SC * PC)
        c0 = c * COLS_PER_CORE
        n = min(COLS_PER_CORE, OW - c0)
        out[:, c0 : c0 + n] = o4[:OH, :n]

    b0 = float(bias.reshape(-1)[0]) if bias.size else 0.0
    if b0 != 0.0:
        out += b0
    return out
